# revision 18
# baseline (speedup 1.0000x reference)
"""Fused FP8-block-quantized MLP (silu(x@w1.T) * (x@w3.T)) @ w2.T on 8 trn2 cores.

Sharding: data-parallel over tokens. Each core gets T/8 = 512 tokens and the
full (dequantized, bf16) weights; there are no collectives. Host-side prep
dequantizes the block-quantized weights, casts to bf16, and lays tensors out
partition-major so every device DMA is one large contiguous transfer.

Device kernel per core (all matmuls bf16, fp32 PSUM accumulation):
  warmup : ~16 dummy matmuls on a zeroed scratch tile keep the PE busy from
           the end of the framework preamble so the HAM clock-gate releases
           (1.2 -> 2.4 GHz) before real data arrives, and real matmuls never
           run throttled.
  phase A: for each 128-row block fb of F: g.T/u.T [128f, 512t] accumulated
           over 16 k-blocks of H, g/u interleaved per kb; silu on ACT, then
           DVE multiplies silu(g) against the u PSUM bank directly -> fusedT
           (bf16) kept in SBUF.
  phase B: out [512t, 2048h] = fusedT.T @ w2.T, streaming w2 column blocks,
           accumulating over the 56 f-blocks in PSUM; bf16 outputs (host
           upcasts to fp32).

DMA: startup x + w13[0] are sliced ~256KB in consumption order and split
across the two HWDGE queues (sync + scalar) to halve issue serialization;
steady-state w13[fb] is two half-DMAs on alternating queues with prefetch
depth 3 so the PE never waits on weights at fb boundaries.
"""

import sys

import numpy as np

_REPO = "/opt/trn_rl_repo"
if _REPO not in sys.path:
    sys.path.insert(0, _REPO)

T, H, F = 4096, 2048, 7168
NCORES = 8
TC = T // NCORES      # 512 tokens per core
KB = H // 128         # 16 contraction blocks for matmul 1/3
FB = F // 128         # 56 f blocks
FB2 = FB // 2         # w2 blocks are streamed in pairs
HCOLS = H // 512      # 4 output column groups
TB = TC // 128        # 4 token blocks
NWARM = 11            # dummy matmuls to pre-release the HAM clock gate
                      # (9 x 427ns cold > the 3.4us HAM activity window, so
                      # the un-throttle fires during warmup, ~10.5us)

_CACHE = {}


def _build_program():
    import concourse.mybir as mybir
    from concourse import bacc
    from concourse.tile import TileContext

    bf16 = mybir.dt.bfloat16
    f32 = mybir.dt.float32

    # Bacc (not bass.Bass): its finalize() runs generate_event_semaphores,
    # which splits multi-wait sync_info into EventSemaphore instructions —
    # TRN2 instructions physically carry at most one sem wait.
    nc = bacc.Bacc()
    # All inputs are laid out partition-major on the host so each DMA below
    # is a single large transfer with contiguous per-partition rows.
    xt_d = nc.declare_dram_parameter("xt", [128, KB, TC], bf16, isOutput=False)
    w13_d = nc.declare_dram_parameter(
        "w13p", [FB, 128, 2, H], bf16, isOutput=False
    )
    w2_d = nc.declare_dram_parameter(
        "w2p", [HCOLS, FB2, 128, 2, 512], bf16, isOutput=False
    )
    out_d = nc.declare_dram_parameter("out", [TC, H], bf16, isOutput=True)

    with TileContext(nc) as tc:
        with (
            tc.tile_pool(name="spool", bufs=1) as spool,
            tc.tile_pool(name="xpool", bufs=1) as xpool,
            tc.tile_pool(name="wpool", bufs=3) as wpool,
            tc.tile_pool(name="w2pool", bufs=8) as w2pool,
            tc.tile_pool(name="sgpool", bufs=3) as sgpool,
            tc.tile_pool(name="fpool", bufs=FB) as fpool,
            tc.tile_pool(name="opool", bufs=HCOLS * TB) as opool,
            tc.tile_pool(name="pp", bufs=8, space="PSUM") as pp,
        ):
            # PE warmup: dense dummy matmuls from the moment the PE clears
            # the framework preamble. They keep the HAM activity window full
            # so the clock un-throttles during the (DMA-bound) fb0 window.
            scratch = spool.tile([128, 512], bf16)
            nc.gpsimd.memset(scratch, 0)
            wps = pp.tile([128, 512], f32, tag="ps", name="warm")
            for _ in range(NWARM):
                nc.tensor.matmul(
                    wps, scratch[:, 0:128], scratch, start=True, stop=True
                )

            xtile = xpool.tile([128, KB, TC], bf16)

            fused = []
            for fb in range(FB):
                w13t = wpool.tile([128, 2, H], bf16, tag="w13t")
                if fb == 0:
                    # Startup, in consumption order with few large DMAs so
                    # neither issue serialization (~0.6us each) nor the 8 DMA
                    # sem lanes bound it. All on the sync queue: the scalar
                    # queue's head is occupied by the two ACT table loads
                    # (~2.6us), which would delay any critical chunk there.
                    # (w13 cols, x kb) chunk pairs sized so no single PE
                    # wait exceeds ~3us — a >3.4us idle window would
                    # re-throttle the clock mid-startup.
                    for wl, wr, xl, xr in (
                        (0, 256, 0, 2),
                        (256, 640, 2, 5),
                        (640, 1024, 5, 8),
                        (1024, 1536, 8, 12),
                        (1536, 2048, 12, 16),
                    ):
                        nc.sync.dma_start(
                            out=w13t[:, :, wl:wr], in_=w13_d[0][:, :, wl:wr]
                        )
                        nc.sync.dma_start(
                            out=xtile[:, xl:xr, :], in_=xt_d[:, xl:xr, :]
                        )
                else:
                    # fb1/fb2 have free pool slots at t=0, so their DMAs
                    # issue immediately — keep them on sync IN ORDER so they
                    # can't steal startup bandwidth from fb0's chunks. fb3+
                    # is gated by slot release (bufs=3), so scalar is safe.
                    h2 = H // 2
                    e2 = nc.sync if fb <= 2 else nc.scalar
                    nc.sync.dma_start(
                        out=w13t[:, :, 0:h2], in_=w13_d[fb][:, :, 0:h2]
                    )
                    e2.dma_start(
                        out=w13t[:, :, h2:H], in_=w13_d[fb][:, :, h2:H]
                    )

                gps = pp.tile([128, TC], f32, tag="ps", name=f"g{fb}")
                ups = pp.tile([128, TC], f32, tag="ps", name=f"u{fb}")
                wfill = (
                    pp.tile([128, TC], f32, tag="ps", name="fill")
                    if fb == 0
                    else None
                )
                # g/u interleaved per kb: during the DMA-bound fb0 window
                # each arriving w13 chunk feeds 2x the matmul work, keeping
                # the PE duty cycle up while the HAM window fills.
                for kb in range(KB):
                    nc.tensor.matmul(
                        gps,
                        w13t[:, 0, kb * 128 : (kb + 1) * 128],
                        xtile[:, kb, :],
                        start=(kb == 0),
                        stop=(kb == KB - 1),
                    )
                    nc.tensor.matmul(
                        ups,
                        w13t[:, 1, kb * 128 : (kb + 1) * 128],
                        xtile[:, kb, :],
                        start=(kb == 0),
                        stop=(kb == KB - 1),
                    )
                    if fb == 0 and kb == 1:
                        # The wait for the x[2:5] chunk here is the one
                        # startup gap >2us; contiguous idle that long
                        # re-throttles the PE clock. Filler matmuls split
                        # the idle below the HAM MID threshold.
                        for _ in range(4):
                            nc.tensor.matmul(
                                wfill,
                                scratch[:, 0:128],
                                scratch,
                                start=True,
                                stop=True,
                            )

                # ACT evacuates the g bank (silu); DVE reads the u bank
                # straight out of PSUM for the multiply, so no ACT copy.
                sg = sgpool.tile([128, TC], f32, tag="sg")
                nc.scalar.activation(
                    sg, gps, mybir.ActivationFunctionType.Silu
                )
                fut = fpool.tile(
                    [128, TC], bf16, tag="fused", name=f"fused{fb}"
                )
                nc.vector.tensor_tensor(
                    fut, sg, ups, mybir.AluOpType.mult
                )
                fused.append(fut)

            for hc in range(HCOLS):
                pss = []
                for tb in range(TB):
                    ps = pp.tile(
                        [128, 512], f32, tag="ps", name=f"pss{hc}_{tb}"
                    )
                    pss.append(ps)
                for j in range(FB2):
                    w2t = w2pool.tile([128, 2, 512], bf16, tag="w2t")
                    nc.sync.dma_start(out=w2t, in_=w2_d[hc, j])
                    for i in range(2):
                        fb = 2 * j + i
                        for tb in range(TB):
                            nc.tensor.matmul(
                                pss[tb],
                                fused[fb][:, tb * 128 : (tb + 1) * 128],
                                w2t[:, i, :],
                                start=(fb == 0),
                                stop=(fb == FB - 1),
                            )
                for tb in range(TB):
                    ot = opool.tile(
                        [128, 512], bf16, tag="ot", name=f"ot{hc}_{tb}"
                    )
                    # Alternate DVE/ACT so the four evacuations drain in
                    # parallel (both ACT table sets stay resident in the two
                    # physical table slots, so Copy never reloads).
                    if tb % 2 == 0:
                        nc.vector.tensor_copy(ot, pss[tb])
                    else:
                        nc.scalar.copy(ot, pss[tb])
                    # Alternate HWDGE queues so the tail out-DMAs don't
                    # serialize on one sequencer.
                    eng = nc.sync if tb % 2 == 0 else nc.scalar
                    eng.dma_start(
                        out=out_d[
                            tb * 128 : (tb + 1) * 128,
                            hc * 512 : (hc + 1) * 512,
                        ],
                        in_=ot,
                    )
    nc.finalize()
    return nc


def _dequant(wq, s):
    wq = np.asarray(wq, dtype=np.float32)
    s = np.asarray(s, dtype=np.float32)
    n, k = wq.shape
    nb, kb = s.shape
    w = wq.reshape(nb, n // nb, kb, k // kb) * s[:, None, :, None]
    return w.reshape(n, k)


def _prep_inputs(hidden_states, w1_q, w1_s, w3_q, w3_s, w2_q, w2_s):
    import ml_dtypes

    bf = ml_dtypes.bfloat16

    w1 = _dequant(w1_q, w1_s).astype(bf)  # [F, H]
    w3 = _dequant(w3_q, w3_s).astype(bf)  # [F, H]
    w2 = _dequant(w2_q, w2_s).astype(bf)  # [H, F]

    # w1p[fb, p, kb*128+c] = w1[fb*128+c, kb*128+p]  (and same for w3);
    # interleaved per partition: w13p[fb, p, 0] = w1 row, [fb, p, 1] = w3.
    w1p = w1.reshape(FB, 128, KB, 128).transpose(0, 3, 2, 1).reshape(FB, 128, H)
    w3p = w3.reshape(FB, 128, KB, 128).transpose(0, 3, 2, 1).reshape(FB, 128, H)
    w13p = np.ascontiguousarray(np.stack([w1p, w3p], axis=2))  # [FB,128,2,H]

    # w2p[hc, j, p, i, c] = w2[hc*512+c, (2j+i)*128+p]
    w2p = np.ascontiguousarray(
        np.asarray(w2).reshape(HCOLS, 512, FB2, 2, 128).transpose(0, 2, 4, 3, 1)
    )

    x = np.asarray(hidden_states, dtype=np.float32).astype(bf)
    xts = []
    for c in range(NCORES):
        xc = x[c * TC : (c + 1) * TC, :]
        # xt[p, kb, t] = xc[t, kb*128+p] — partition-major, so the whole
        # 2MB x-transpose lands in one DMA with 16KB/partition contiguous.
        xts.append(
            np.ascontiguousarray(xc.reshape(TC, KB, 128).transpose(2, 1, 0))
        )

    return [
        {"xt": xts[c], "w13p": w13p, "w2p": w2p}
        for c in range(NCORES)
    ]


def _run(in_maps, **kwargs):
    from concourse.bass_utils import run_bass_kernel_spmd

    if "nc" not in _CACHE:
        _CACHE["nc"] = _build_program()
    res = run_bass_kernel_spmd(
        _CACHE["nc"], in_maps, list(range(NCORES)), **kwargs
    )
    out = np.concatenate(
        [np.asarray(res.results[c]["out"], dtype=np.float32) for c in range(NCORES)],
        axis=0,
    )
    return out, res


def kernel(hidden_states, w1_q, w1_s, w3_q, w3_s, w2_q, w2_s):
    in_maps = _prep_inputs(
        hidden_states, w1_q, w1_s, w3_q, w3_s, w2_q, w2_s
    )
    out, _ = _run(in_maps)
    return out
